# revision 12
# baseline (speedup 1.0000x reference)
"""Trainium2 Bass kernel for nn_LossTDSurv (survival loss over hazards).

Strategy: pure data-parallel over 8 cores, plus HOST-side row grouping.
The loss is row-permutation invariant, and sharding is free-form, so the
host deals the B=524288 rows into 8 cores x 64 groups, where group v
holds only rows with idx == v (fixed 1152-row slots, padded with dummy
rows h=1e-30, e=0 whose contribution to every partial sum is exactly 0).

On device, idx is then a COMPILE-TIME constant per group, so the three
data-dependent quantities per row become static-slice operations:
   A  = sum_{k<=v-2} log(1-h_k)   -> tensor_reduce over lg[:, :, :v-1]
   C  = A + lg[v-1] + lg[v]       -> two small adds
   hv = h[v], lgv = lg[v]         -> strided copies
No gather / scan / masks / GPSIMD anywhere.  The kernel is a plain
DMA -> ACT(Ln) -> reduce pipeline at the HBM roofline.

Per-core output: [128, 6] fp32 partial sums; host combines in float64:
   loss = 0.5*L_z + 0.5*L_c + 1.0*nll
"""

import numpy as np

B_TOTAL = 524288
T = 64
N_CORES = 8
G = 64                 # idx groups
JB = 9                 # row-blocks of 128 per group -> 1152 rows/group
GR = 128 * JB          # rows per group slot
RPC = G * GR           # padded rows per core = 73728
GPST = 4               # groups per supertile (ACT batching)
H_DUMMY = 1e-30
LOG_CLIP = float(np.log(np.float32(1e-8)))

_CACHE = {}


def _build_nc(jb=JB, gpst=GPST):
    """Single-core SPMD Bass program (same NEFF on all 8 cores)."""
    import concourse.bacc as bacc
    import concourse.mybir as mybir
    import concourse.tile as tile

    gr = 128 * jb
    fwg = jb * T                    # free width of one group = jb*64
    nbuf = G * jb                   # per-row buffer width = 576
    f32 = mybir.dt.float32
    AF = mybir.ActivationFunctionType
    OP = mybir.AluOpType
    AX = mybir.AxisListType

    nc = bacc.Bacc("TRN2", target_bir_lowering=False, debug=False)

    hsort = nc.dram_tensor("hsort", [G * gr, T], f32, kind="ExternalInput")
    esort = nc.dram_tensor("esort", [128, nbuf], f32, kind="ExternalInput")
    partials = nc.dram_tensor("partials", [128, 6], f32, kind="ExternalOutput")

    with tile.TileContext(nc) as tc:
        with (
            tc.tile_pool(name="io", bufs=3) as io,
            tc.tile_pool(name="work", bufs=2) as work,
            tc.tile_pool(name="pers", bufs=1) as pers,
        ):
            Ab = pers.tile([128, nbuf], f32, tag="Ab")
            Cb = pers.tile([128, nbuf], f32, tag="Cb")
            Hb = pers.tile([128, nbuf], f32, tag="Hb")
            Lv = pers.tile([128, nbuf], f32, tag="Lv")
            Eb = pers.tile([128, nbuf], f32, tag="Eb")
            nc.sync.dma_start(Eb[:], esort[:])

            n_st = G // gpst
            for st in range(n_st):
                g0 = st * gpst
                # contiguous [gpst*gr rows, 64] -> [128, gpst*fwg]
                Wt = io.tile([128, gpst * fwg], f32, tag="W")
                hview = hsort[g0 * gr:(g0 + gpst) * gr, :].rearrange(
                    "(g p j) t -> p g (j t)", p=128, g=gpst
                )
                nc.sync.dma_start(
                    Wt[:].rearrange("p (g f) -> p g f", g=gpst), hview
                )
                lg = work.tile([128, gpst * fwg], f32, tag="lg")
                nc.scalar.activation(lg[:], Wt[:], AF.Ln, bias=1.0, scale=-1.0)

                lg4 = lg[:].rearrange("p (g j t) -> p g j t", g=gpst, t=T)
                w4 = Wt[:].rearrange("p (g j t) -> p g j t", g=gpst, t=T)
                for gi in range(gpst):
                    v = g0 + gi
                    sl = slice(v * jb, (v + 1) * jb)
                    if v >= 2:
                        nc.vector.tensor_reduce(
                            Ab[:, sl], lg4[:, gi, :, :v - 1], axis=AX.X, op=OP.add
                        )
                    else:
                        nc.vector.memset(Ab[:, sl], 0.0)
                    # C = A + lg[v-1] + lg[v]
                    if v == 0:
                        nc.vector.tensor_copy(Cb[:, sl], lg4[:, gi, :, 0])
                    elif v == 1:
                        nc.vector.tensor_tensor(
                            out=Cb[:, sl], in0=lg4[:, gi, :, 0],
                            in1=lg4[:, gi, :, 1], op=OP.add,
                        )
                    else:
                        nc.vector.tensor_tensor(
                            out=Cb[:, sl], in0=Ab[:, sl],
                            in1=lg4[:, gi, :, v - 1], op=OP.add,
                        )
                        nc.vector.tensor_tensor(
                            out=Cb[:, sl], in0=Cb[:, sl],
                            in1=lg4[:, gi, :, v], op=OP.add,
                        )
                    nc.scalar.copy(Hb[:, sl], w4[:, gi, :, v])
                    nc.scalar.copy(Lv[:, sl], lg4[:, gi, :, v])

            # ---------------- epilogue over [128, nbuf] ----------------
            ep = pers.tile([128, nbuf], f32, tag="ep")
            ep2 = pers.tile([128, nbuf], f32, tag="ep2")
            acc = pers.tile([128, 6], f32, tag="acc")
            loghv = pers.tile([128, nbuf], f32, tag="loghv")
            nc.scalar.activation(loghv[:], Hb[:], AF.Ln)

            # s0 = sum e*(loghv + A)
            nc.vector.tensor_tensor(out=ep[:], in0=loghv[:], in1=Ab[:], op=OP.add)
            nc.vector.scalar_tensor_tensor(
                out=ep2[:], in0=ep[:], scalar=0.0, in1=Eb[:],
                op0=OP.add, op1=OP.mult, accum_out=acc[:, 0:1],
            )
            # s1 = sum e
            nc.vector.tensor_reduce(acc[:, 1:2], Eb[:], axis=AX.X, op=OP.add)
            # censoring: s2 = sum A ; s3 = sum e*(logwt - A)
            expa = ep
            nc.scalar.activation(expa[:], Ab[:], AF.Exp)
            wt = ep2
            nc.vector.tensor_scalar(
                out=wt[:], in0=expa[:], scalar1=-1.0, scalar2=1.0,
                op0=OP.mult, op1=OP.add,
            )  # 1 - exp(A)
            nc.vector.tensor_scalar_max(out=wt[:], in0=wt[:], scalar1=1e-8)
            logwt = pers.tile([128, nbuf], f32, tag="logwt")
            nc.scalar.activation(logwt[:], wt[:], AF.Ln)
            # groups v=0,1: reference gives log(1e-8) exactly
            nc.vector.memset(logwt[:, 0:2 * jb], LOG_CLIP)
            nc.vector.tensor_reduce(acc[:, 2:3], Ab[:], axis=AX.X, op=OP.add)
            nc.vector.tensor_tensor(out=ep[:], in0=logwt[:], in1=Ab[:],
                                    op=OP.subtract)
            nc.vector.scalar_tensor_tensor(
                out=ep2[:], in0=ep[:], scalar=0.0, in1=Eb[:],
                op0=OP.add, op1=OP.mult, accum_out=acc[:, 3:4],
            )
            # nll: s4 = sum C ; s5 = sum e*phi, phi = loghv - lgv
            nc.vector.tensor_reduce(acc[:, 4:5], Cb[:], axis=AX.X, op=OP.add)
            nc.vector.tensor_tensor(out=ep[:], in0=loghv[:], in1=Lv[:],
                                    op=OP.subtract)
            nc.vector.scalar_tensor_tensor(
                out=ep2[:], in0=ep[:], scalar=0.0, in1=Eb[:],
                op0=OP.add, op1=OP.mult, accum_out=acc[:, 5:6],
            )

            nc.sync.dma_start(partials[:], acc[:])

    nc.finalize()
    return nc


def _pack_core(preds_rows, e_rows, idx_rows, jb=JB):
    """Pack one core's rows into the grouped layout.

    Returns hsort [G*gr, T] and esort [128, G*jb]."""
    gr = 128 * jb
    hsort = np.full((G * gr, T), H_DUMMY, np.float32)
    e_slot = np.zeros(G * gr, np.float32)
    for v in range(G):
        m = idx_rows == v
        n = int(m.sum())
        assert n <= gr, f"group {v} overflow: {n} > {gr}"
        hsort[v * gr:v * gr + n] = preds_rows[m]
        e_slot[v * gr:v * gr + n] = e_rows[m]
    esort = (
        e_slot.reshape(G, 128, jb).transpose(1, 0, 2).reshape(128, G * jb)
    )
    return hsort, np.ascontiguousarray(esort)


def _combine(partials_list, b_total):
    s = np.zeros(6, np.float64)
    for pcore in partials_list:
        s += pcore.astype(np.float64).sum(axis=0)
    s_eu, s_e, s_a, s_ed, s_c, s_ephi = s
    L_z = -s_eu / s_e
    L_c = -(s_a + s_ed) / b_total
    nll = -(s_c + s_ephi) / b_total
    return np.float32(0.5 * L_z + 0.5 * L_c + 1.0 * nll)


def kernel(preds: np.ndarray, target: np.ndarray) -> np.ndarray:
    from concourse.bass_utils import run_bass_kernel_spmd

    preds = np.asarray(preds, np.float32).reshape(B_TOTAL, T)
    target = np.asarray(target, np.float32).reshape(B_TOTAL, 3)
    idx = target[:, 0].astype(np.int64)
    ev = target[:, 1].astype(np.float32)

    if "nc" not in _CACHE:
        _CACHE["nc"] = _build_nc()
    nc = _CACHE["nc"]

    # deal rows round-robin across cores (keeps every per-core idx-group
    # below its fixed 1152-row slot with overwhelming probability)
    in_maps = []
    for c in range(N_CORES):
        m = (np.arange(B_TOTAL) % N_CORES) == c
        hs, es = _pack_core(preds[m], ev[m], idx[m])
        in_maps.append({"hsort": hs, "esort": es})

    res = run_bass_kernel_spmd(nc, in_maps, core_ids=list(range(N_CORES)))
    _CACHE["last_results"] = res
    return _combine([r["partials"] for r in res.results], float(B_TOTAL))


if __name__ == "__main__":
    pass


# revision 13
# speedup vs baseline: 1.0290x; 1.0290x over previous
"""Trainium2 Bass kernel for nn_LossTDSurv (survival loss over hazards).

Strategy: pure data-parallel over 8 cores, plus HOST-side row grouping.
The loss is row-permutation invariant, and sharding is free-form, so the
host deals the B=524288 rows into 8 cores x 64 groups, where group v
holds only rows with idx == v (fixed 1152-row slots, padded with dummy
rows h=1e-30, e=0 whose contribution to every partial sum is exactly 0).

On device, idx is then a COMPILE-TIME constant per group, so the three
data-dependent quantities per row become static-slice operations:
   A  = sum_{k<=v-2} log(1-h_k)   -> tensor_reduce over lg[:, :, :v-1]
   C  = A + lg[v-1] + lg[v]       -> two small adds
   hv = h[v], lgv = lg[v]         -> strided copies
No gather / scan / masks / GPSIMD anywhere.  The kernel is a plain
DMA -> ACT(Ln) -> reduce pipeline at the HBM roofline.

Per-core output: [128, 6] fp32 partial sums; host combines in float64:
   loss = 0.5*L_z + 0.5*L_c + 1.0*nll
"""

import numpy as np

B_TOTAL = 524288
T = 64
N_CORES = 8
G = 64                 # idx groups
JB = 9                 # row-blocks of 128 per group -> 1152 rows/group
GR = 128 * JB          # rows per group slot
RPC = G * GR           # padded rows per core = 73728
GPST = 4               # groups per supertile (ACT batching)
H_DUMMY = 1e-30
LOG_CLIP = float(np.log(np.float32(1e-8)))

_CACHE = {}


def _build_nc(jb=JB, gpst=GPST):
    """Single-core SPMD Bass program (same NEFF on all 8 cores)."""
    import concourse.bacc as bacc
    import concourse.mybir as mybir
    import concourse.tile as tile

    gr = 128 * jb
    fwg = jb * T                    # free width of one group = jb*64
    nbuf = G * jb                   # per-row buffer width = 576
    f32 = mybir.dt.float32
    AF = mybir.ActivationFunctionType
    OP = mybir.AluOpType
    AX = mybir.AxisListType

    nc = bacc.Bacc("TRN2", target_bir_lowering=False, debug=False)

    hsort = nc.dram_tensor("hsort", [G * gr, T], f32, kind="ExternalInput")
    esort = nc.dram_tensor("esort", [128, nbuf], f32, kind="ExternalInput")
    partials = nc.dram_tensor("partials", [128, 6], f32, kind="ExternalOutput")

    with tile.TileContext(nc) as tc:
        with (
            tc.tile_pool(name="io", bufs=3) as io,
            tc.tile_pool(name="work", bufs=2) as work,
            tc.tile_pool(name="pers", bufs=1) as pers,
        ):
            Ab = pers.tile([128, nbuf], f32, tag="Ab")
            Cb = pers.tile([128, nbuf], f32, tag="Cb")
            Hb = pers.tile([128, nbuf], f32, tag="Hb")
            Eb = pers.tile([128, nbuf], f32, tag="Eb")
            nc.sync.dma_start(Eb[:], esort[:])

            n_st = G // gpst
            for st in range(n_st):
                g0 = st * gpst
                # contiguous [gpst*gr rows, 64] -> [128, gpst*fwg]
                Wt = io.tile([128, gpst * fwg], f32, tag="W")
                hview = hsort[g0 * gr:(g0 + gpst) * gr, :].rearrange(
                    "(g p j) t -> p g (j t)", p=128, g=gpst
                )
                nc.sync.dma_start(
                    Wt[:].rearrange("p (g f) -> p g f", g=gpst), hview
                )
                lg = work.tile([128, gpst * fwg], f32, tag="lg")
                nc.scalar.activation(lg[:], Wt[:], AF.Ln, bias=1.0, scale=-1.0)

                lg4 = lg[:].rearrange("p (g j t) -> p g j t", g=gpst, t=T)
                w4 = Wt[:].rearrange("p (g j t) -> p g j t", g=gpst, t=T)
                for gi in range(gpst):
                    v = g0 + gi
                    sl = slice(v * jb, (v + 1) * jb)
                    if v >= 2:
                        nc.vector.tensor_reduce(
                            Ab[:, sl], lg4[:, gi, :, :v - 1], axis=AX.X, op=OP.add
                        )
                    else:
                        nc.vector.memset(Ab[:, sl], 0.0)
                    # C = A + lg[v-1] + lg[v]
                    if v == 0:
                        nc.vector.tensor_copy(Cb[:, sl], lg4[:, gi, :, 0])
                    elif v == 1:
                        nc.vector.tensor_tensor(
                            out=Cb[:, sl], in0=lg4[:, gi, :, 0],
                            in1=lg4[:, gi, :, 1], op=OP.add,
                        )
                    else:
                        nc.vector.tensor_tensor(
                            out=Cb[:, sl], in0=Ab[:, sl],
                            in1=lg4[:, gi, :, v - 1], op=OP.add,
                        )
                        nc.vector.tensor_tensor(
                            out=Cb[:, sl], in0=Cb[:, sl],
                            in1=lg4[:, gi, :, v], op=OP.add,
                        )
                    nc.vector.tensor_copy(Hb[:, sl], w4[:, gi, :, v])

            # ---------------- epilogue over [128, nbuf] ----------------
            ep = pers.tile([128, nbuf], f32, tag="ep")
            ep2 = pers.tile([128, nbuf], f32, tag="ep2")
            acc = pers.tile([128, 6], f32, tag="acc")
            loghv = pers.tile([128, nbuf], f32, tag="loghv")
            nc.scalar.activation(loghv[:], Hb[:], AF.Ln)

            # s0 = sum e*(loghv + A)
            nc.vector.tensor_tensor(out=ep[:], in0=loghv[:], in1=Ab[:], op=OP.add)
            nc.vector.scalar_tensor_tensor(
                out=ep2[:], in0=ep[:], scalar=0.0, in1=Eb[:],
                op0=OP.add, op1=OP.mult, accum_out=acc[:, 0:1],
            )
            # s1 = sum e
            nc.vector.tensor_reduce(acc[:, 1:2], Eb[:], axis=AX.X, op=OP.add)
            # censoring: s2 = sum A ; s3 = sum e*(logwt - A)
            expa = ep
            nc.scalar.activation(expa[:], Ab[:], AF.Exp)
            wt = ep2
            nc.vector.tensor_scalar(
                out=wt[:], in0=expa[:], scalar1=-1.0, scalar2=1.0,
                op0=OP.mult, op1=OP.add,
            )  # 1 - exp(A)
            nc.vector.tensor_scalar_max(out=wt[:], in0=wt[:], scalar1=1e-8)
            logwt = pers.tile([128, nbuf], f32, tag="logwt")
            nc.scalar.activation(logwt[:], wt[:], AF.Ln)
            # groups v=0,1: reference gives log(1e-8) exactly
            nc.vector.memset(logwt[:, 0:2 * jb], LOG_CLIP)
            nc.vector.tensor_reduce(acc[:, 2:3], Ab[:], axis=AX.X, op=OP.add)
            nc.vector.tensor_tensor(out=ep[:], in0=logwt[:], in1=Ab[:],
                                    op=OP.subtract)
            nc.vector.scalar_tensor_tensor(
                out=ep2[:], in0=ep[:], scalar=0.0, in1=Eb[:],
                op0=OP.add, op1=OP.mult, accum_out=acc[:, 3:4],
            )
            # nll: s4 = sum C ; s5 = sum e*phi, phi = loghv - ln(1-hv)
            nc.vector.tensor_reduce(acc[:, 4:5], Cb[:], axis=AX.X, op=OP.add)
            lgv = pers.tile([128, nbuf], f32, tag="lgv")
            nc.scalar.activation(lgv[:], Hb[:], AF.Ln, bias=1.0, scale=-1.0)
            nc.vector.tensor_tensor(out=ep[:], in0=loghv[:], in1=lgv[:],
                                    op=OP.subtract)
            nc.vector.scalar_tensor_tensor(
                out=ep2[:], in0=ep[:], scalar=0.0, in1=Eb[:],
                op0=OP.add, op1=OP.mult, accum_out=acc[:, 5:6],
            )

            nc.sync.dma_start(partials[:], acc[:])

    nc.finalize()
    return nc


def _pack_core(preds_rows, e_rows, idx_rows, jb=JB):
    """Pack one core's rows into the grouped layout.

    Returns hsort [G*gr, T] and esort [128, G*jb]."""
    gr = 128 * jb
    hsort = np.full((G * gr, T), H_DUMMY, np.float32)
    e_slot = np.zeros(G * gr, np.float32)
    for v in range(G):
        m = idx_rows == v
        n = int(m.sum())
        assert n <= gr, f"group {v} overflow: {n} > {gr}"
        hsort[v * gr:v * gr + n] = preds_rows[m]
        e_slot[v * gr:v * gr + n] = e_rows[m]
    esort = (
        e_slot.reshape(G, 128, jb).transpose(1, 0, 2).reshape(128, G * jb)
    )
    return hsort, np.ascontiguousarray(esort)


def _combine(partials_list, b_total):
    s = np.zeros(6, np.float64)
    for pcore in partials_list:
        s += pcore.astype(np.float64).sum(axis=0)
    s_eu, s_e, s_a, s_ed, s_c, s_ephi = s
    L_z = -s_eu / s_e
    L_c = -(s_a + s_ed) / b_total
    nll = -(s_c + s_ephi) / b_total
    return np.float32(0.5 * L_z + 0.5 * L_c + 1.0 * nll)


def kernel(preds: np.ndarray, target: np.ndarray) -> np.ndarray:
    from concourse.bass_utils import run_bass_kernel_spmd

    preds = np.asarray(preds, np.float32).reshape(B_TOTAL, T)
    target = np.asarray(target, np.float32).reshape(B_TOTAL, 3)
    idx = target[:, 0].astype(np.int64)
    ev = target[:, 1].astype(np.float32)

    if "nc" not in _CACHE:
        _CACHE["nc"] = _build_nc()
    nc = _CACHE["nc"]

    # deal rows round-robin across cores (keeps every per-core idx-group
    # below its fixed 1152-row slot with overwhelming probability)
    in_maps = []
    for c in range(N_CORES):
        m = (np.arange(B_TOTAL) % N_CORES) == c
        hs, es = _pack_core(preds[m], ev[m], idx[m])
        in_maps.append({"hsort": hs, "esort": es})

    res = run_bass_kernel_spmd(nc, in_maps, core_ids=list(range(N_CORES)))
    _CACHE["last_results"] = res
    return _combine([r["partials"] for r in res.results], float(B_TOTAL))


if __name__ == "__main__":
    pass
